# revision 7
# baseline (speedup 1.0000x reference)
"""Trainium2 Bass kernel for ContinuousMessagePassing (GNN message passing).

Math (per reference):
    h   = relu(x @ W1.T)                 # [N, 256]
    m   = relu(h @ W2.T)                 # [N, 128]
    y   = segment_mean(m[src], dst, N)   # [N, 128]  (0 for isolated nodes)
    gi  = [x, y] @ W_ih.T ; gh = z @ W_hh.T
    r, u = sigmoid(gi_r + gh_r), sigmoid(gi_u + gh_u)
    n   = tanh(gi_n + r * gh_n)
    out = (1 - u) * n + u * z

Distribution: nodes sharded across 8 cores; the m-table computation is
replicated on every core, but split into S=4 source segments so that the
edge gather/scatter for segment s overlaps the computation of segment s+1.
Each core processes the edges whose dst lands in its shard (host buckets,
sorts per (segment, dst-tile, src) and pads to a shared static schedule).

All matmul operands are pre-transposed on the host (x.T, z.T) so the device
never runs tensor-engine transposes for layout; the GRU gates are computed
with 512-wide fused r|u matmuls.  Biases are all zeros per the problem spec
and are folded out.
"""

import numpy as np
import ml_dtypes

BF16 = ml_dtypes.bfloat16

# ---------------------------------------------------------------- config

P = 128          # partitions
CHUNK = 512      # nodes per matmul chunk
S = 4            # source segments (phase A/B interleave granularity)
GT = 8           # edge-tiles (of 128 edges) per dma_gather call
NQ = 4           # SWDGE queues
MSG_BUFS = 12    # gather groups in flight

N_REAL = 50000
N_CORES = 8
SHARD_REAL = 6250
SHARD_PAD = 6656          # 52 tiles of 128
NPAD = N_CORES * SHARD_PAD    # 53248
SEG = NPAD // S               # 13312 rows per source segment (int16-safe)
NT = SHARD_PAD // P           # 52 node tiles per shard
IN_F = 256
MSG = 128
HID = 256
OUT_F = 256


class _Cfg:  # kept for test.py compatibility
    n_cores = N_CORES
    n_real = N_REAL
    shard_real = SHARD_REAL
    shard_pad = SHARD_PAD
    npad = NPAD
    nt = NT


CFG8 = _Cfg()


# ---------------------------------------------------------------- host prep

def _wrap_idx16(idx_flat):
    """[n] int array -> [128, n//16] int16 in the dma_gather layout:
    position i lives at [i % 16, i // 16], replicated across the 8 groups
    of 16 partitions (one copy per Q7 core)."""
    n = idx_flat.shape[0]
    a = np.ascontiguousarray(idx_flat.reshape(n // 16, 16).T).astype(np.int16)
    return np.ascontiguousarray(np.tile(a, (8, 1)))


def _prep(inputs):
    x = np.asarray(inputs["x"], np.float32)
    z = np.asarray(inputs["z"], np.float32)
    src = np.asarray(inputs["src"], np.int64)
    dst = np.asarray(inputs["dst"], np.int64)

    # padded node arrays, shard k at rows [k*SHARD_PAD, k*SHARD_PAD+SHARD_REAL)
    xp = np.zeros((NPAD, IN_F), dtype=BF16)
    zp = np.zeros((NPAD, OUT_F), dtype=np.float32)
    for k in range(N_CORES):
        xp[k * SHARD_PAD: k * SHARD_PAD + SHARD_REAL] = x[
            k * SHARD_REAL: (k + 1) * SHARD_REAL]
        zp[k * SHARD_PAD: k * SHARD_PAD + SHARD_REAL] = z[
            k * SHARD_REAL: (k + 1) * SHARD_REAL]
    xt = np.ascontiguousarray(xp.T)                       # [256, NPAD] bf16

    w1t = np.ascontiguousarray(np.asarray(inputs["W1"], np.float32).T).astype(BF16)
    w2t = np.ascontiguousarray(np.asarray(inputs["W2"], np.float32).T).astype(BF16)
    wiht = np.ascontiguousarray(np.asarray(inputs["W_ih"], np.float32).T).astype(BF16)
    whht = np.ascontiguousarray(np.asarray(inputs["W_hh"], np.float32).T).astype(BF16)

    src_pad = (src // SHARD_REAL) * SHARD_PAD + src % SHARD_REAL
    seg = src_pad // SEG
    idx_in_seg = src_pad % SEG
    owner = dst // SHARD_REAL
    dloc = dst - owner * SHARD_REAL
    tile_id = dloc // P
    rel = dloc % P

    # per (core, seg, tile) counts -> shared schedule T[s][t]
    cnt = np.zeros((N_CORES, S, NT), np.int64)
    for k in range(N_CORES):
        sel = owner == k
        np.add.at(cnt[k], (seg[sel], tile_id[sel]), 1)
    T = ((cnt.max(axis=0) + P - 1) // P).astype(np.int64)     # [S, NT]
    # tiles past the real-node region (SHARD_REAL/P) legitimately have no
    # edges; their y is garbage and their outputs are discarded pad rows
    assert (T[:, : SHARD_REAL // P] > 0).all()
    for s in range(S):
        T[s, -1] += (-int(T[s].sum())) % GT
    stot = [int(T[s].sum()) for s in range(S)]

    in_maps = []
    for k in range(N_CORES):
        sel = np.nonzero(owner == k)[0]
        order = np.lexsort((idx_in_seg[sel], tile_id[sel], seg[sel]))
        esel = sel[order]
        sseg = seg[esel]
        stid = tile_id[esel]

        idx16_s, rel_s = [], []
        for s in range(S):
            idx_stream = np.zeros(stot[s] * P, np.int64)
            rel_stream = np.full(stot[s] * P, -1.0, np.float32)
            off = 0
            for t in range(NT):
                e = esel[(sseg == s) & (stid == t)]
                c = e.shape[0]
                idx_stream[off: off + c] = idx_in_seg[e]
                rel_stream[off: off + c] = rel[e]
                off += int(T[s, t]) * P
            blocks = [
                _wrap_idx16(idx_stream[g * GT * P: (g + 1) * GT * P])
                for g in range(stot[s] // GT)
            ]
            idx16_s.append(np.concatenate(blocks, axis=1))
            rel_s.append(np.ascontiguousarray(
                rel_stream.reshape(stot[s], P).T).astype(BF16))

        cnt_nodes = np.bincount(dloc[owner == k], minlength=SHARD_PAD)
        inv = (1.0 / np.maximum(cnt_nodes, 1)).astype(np.float32)
        inv2 = np.ascontiguousarray(inv.reshape(NT, P).T)          # [128, NT]

        r0 = k * SHARD_PAD
        im = {
            "xt": xt,
            "xt_own": np.ascontiguousarray(xt[:, r0: r0 + SHARD_PAD]),
            "zt_own": np.ascontiguousarray(zp[r0: r0 + SHARD_PAD].T).astype(BF16),
            "z_own": zp[r0: r0 + SHARD_PAD],
            "w1t": w1t,
            "w2t": w2t,
            "wiht": wiht,
            "whht": whht,
            "invcnt": inv2,
            "iota_c": np.tile(np.arange(P, dtype=np.float32), (P, 1)).astype(BF16),
            "ident_bf": np.eye(P, dtype=np.float32).astype(BF16),
        }
        for s in range(S):
            im[f"idx_s{s}"] = idx16_s[s]
            im[f"rel_s{s}"] = rel_s[s]
        in_maps.append(im)
    return in_maps, T


# ---------------------------------------------------------------- device program

def _build(T, debug=False):
    import concourse.bass as bass  # noqa: F401
    import concourse.tile as tile
    from concourse import bacc, mybir

    dt = mybir.dt
    Act = mybir.ActivationFunctionType
    Alu = mybir.AluOpType

    stot = [int(T[s].sum()) for s in range(S)]
    CPS = (NPAD // CHUNK) // S          # phase-A chunks per segment (26)

    nc = bacc.Bacc(None, num_swdge_queues=NQ, dynamic_dma_scratch_size=65536)

    xt_d = nc.dram_tensor("xt", [IN_F, NPAD], dt.bfloat16, kind="ExternalInput")
    xt_own_d = nc.dram_tensor("xt_own", [IN_F, SHARD_PAD], dt.bfloat16, kind="ExternalInput")
    zt_own_d = nc.dram_tensor("zt_own", [OUT_F, SHARD_PAD], dt.bfloat16, kind="ExternalInput")
    z_own_d = nc.dram_tensor("z_own", [SHARD_PAD, OUT_F], dt.float32, kind="ExternalInput")
    w1t_d = nc.dram_tensor("w1t", [IN_F, HID], dt.bfloat16, kind="ExternalInput")
    w2t_d = nc.dram_tensor("w2t", [HID, MSG], dt.bfloat16, kind="ExternalInput")
    wiht_d = nc.dram_tensor("wiht", [IN_F + MSG, 3 * OUT_F], dt.bfloat16, kind="ExternalInput")
    whht_d = nc.dram_tensor("whht", [OUT_F, 3 * OUT_F], dt.bfloat16, kind="ExternalInput")
    idx_d = [nc.dram_tensor(f"idx_s{s}", [P, stot[s] * 8], dt.int16, kind="ExternalInput")
             for s in range(S)]
    rel_d = [nc.dram_tensor(f"rel_s{s}", [P, stot[s]], dt.bfloat16, kind="ExternalInput")
             for s in range(S)]
    inv_d = nc.dram_tensor("invcnt", [P, NT], dt.float32, kind="ExternalInput")
    iota_d = nc.dram_tensor("iota_c", [P, P], dt.bfloat16, kind="ExternalInput")
    identb_d = nc.dram_tensor("ident_bf", [P, P], dt.bfloat16, kind="ExternalInput")
    hout = nc.dram_tensor("hout", [SHARD_PAD, OUT_F], dt.float32, kind="ExternalOutput")
    if debug:
        dbg_yt = nc.dram_tensor("dbg_yt", [P, SHARD_PAD], dt.bfloat16, kind="ExternalOutput")
    # m table, 256-elem (512B) bf16 rows, split in S segment tensors so the
    # segment-s gathers only depend on segment-s writes
    m_seg = [nc.dram_tensor(f"m_seg{s}", [SEG, 2 * MSG], dt.bfloat16) for s in range(S)]

    with tile.TileContext(nc) as tc:
        with tc.tile_pool(name="persist", bufs=1) as pers:
            w1t_sb = pers.tile([P, 2, HID], dt.bfloat16)
            nc.sync.dma_start(w1t_sb[:], w1t_d[:, :].rearrange("(k p) n -> p k n", p=P))
            w2t_sb = pers.tile([P, 2, MSG], dt.bfloat16)
            nc.sync.dma_start(w2t_sb[:], w2t_d[:, :].rearrange("(k p) n -> p k n", p=P))
            wiht_sb = pers.tile([P, 3, 3 * OUT_F], dt.bfloat16)
            nc.sync.dma_start(wiht_sb[:], wiht_d[:, :].rearrange("(k p) n -> p k n", p=P))
            whht_sb = pers.tile([P, 2, 3 * OUT_F], dt.bfloat16)
            nc.sync.dma_start(whht_sb[:], whht_d[:, :].rearrange("(k p) n -> p k n", p=P))
            idx_sb = []
            rel_sb = []
            for s in range(S):
                isb = pers.tile([P, stot[s] * 8], dt.int16)
                nc.sync.dma_start(isb[:], idx_d[s][:, :])
                idx_sb.append(isb)
                rsb = pers.tile([P, stot[s]], dt.bfloat16)
                nc.sync.dma_start(rsb[:], rel_d[s][:, :])
                rel_sb.append(rsb)
            inv_sb = pers.tile([P, NT], dt.float32)
            nc.sync.dma_start(inv_sb[:], inv_d[:, :])
            iota_sb = pers.tile([P, P], dt.bfloat16)
            nc.sync.dma_start(iota_sb[:], iota_d[:, :])
            ident_bf = pers.tile([P, P], dt.bfloat16)
            nc.sync.dma_start(ident_bf[:], identb_d[:, :])

            y_acc = pers.tile([P, SHARD_PAD], dt.float32)    # per-dst partial sums
            yT_own = pers.tile([P, SHARD_PAD], dt.bfloat16)  # y transposed for GRU

            with (
                tc.tile_pool(name="pa", bufs=2) as pa,
                tc.tile_pool(name="pap", bufs=2, space="PSUM") as pap,
                tc.tile_pool(name="pb", bufs=MSG_BUFS) as pb,
                tc.tile_pool(name="pbo", bufs=8) as pbo,
                tc.tile_pool(name="pbp", bufs=2, space="PSUM") as pbp,
            ):
                # ---------------- phase A: m-table for one source segment ----
                def emit_A(s):
                    for cc in range(CPS):
                        c = s * CPS + cc
                        xT = pa.tile([P, 2, CHUNK], dt.bfloat16, tag="xT")
                        nc.sync.dma_start(
                            xT[:],
                            xt_d.rearrange("(k p) n -> p k n", p=P)[
                                :, :, c * CHUNK: (c + 1) * CHUNK],
                        )
                        hT = pa.tile([P, 2, CHUNK], dt.bfloat16, tag="hT")
                        for mh in range(2):
                            h_ps = pap.tile([P, CHUNK], dt.float32, tag="hp")
                            for kk in range(2):
                                nc.tensor.matmul(
                                    h_ps[:],
                                    lhsT=w1t_sb[:, kk, mh * P: (mh + 1) * P],
                                    rhs=xT[:, kk, :],
                                    start=(kk == 0), stop=(kk == 1),
                                )
                            nc.scalar.activation(hT[:, mh, :], h_ps[:], Act.Relu)
                        m_sb = pa.tile([P, CHUNK // P, 2 * MSG], dt.bfloat16, tag="m_sb")
                        m_ps = pap.tile([P, CHUNK // P, MSG], dt.float32, tag="mp")
                        for t4 in range(CHUNK // P):
                            for kk in range(2):
                                nc.tensor.matmul(
                                    m_ps[:, t4, :],
                                    lhsT=hT[:, kk, t4 * P: (t4 + 1) * P],
                                    rhs=w2t_sb[:, kk, :],
                                    start=(kk == 0), stop=(kk == 1),
                                )
                            nc.scalar.activation(
                                m_sb[:, t4, 0:MSG], m_ps[:, t4, :], Act.Relu)
                        nc.sync.dma_start(
                            m_seg[s][cc * CHUNK: (cc + 1) * CHUNK, :].rearrange(
                                "(t p) f -> p t f", p=P),
                            m_sb[:],
                        )

                # ---------------- phase B: gather + segment reduce, one pass --
                gq = [0]

                def emit_B(s):
                    m_ap = m_seg[s][:, :]
                    isb = idx_sb[s]
                    rsb = rel_sb[s]
                    state = {"et": 0, "msgs": None}

                    def consume():
                        et = state["et"]
                        g, slot = divmod(et, GT)
                        if slot == 0:
                            msgs = pb.tile([P, GT, 2 * MSG], dt.bfloat16,
                                           tag="msgs")
                            nc.gpsimd.dma_gather(
                                msgs[:], m_ap,
                                isb[:, g * GT * 8: (g + 1) * GT * 8],
                                GT * P, GT * P, 2 * MSG,
                                queue_num=gq[0] % NQ,
                            )
                            gq[0] += 1
                            state["msgs"] = msgs
                        state["et"] = et + 1
                        return state["msgs"][:, slot, 0:MSG], et

                    for t in range(NT):
                        total = int(T[s, t])
                        if total == 0:
                            continue
                        ps = pbp.tile([P, MSG], dt.float32, tag="ps")
                        for j in range(total):
                            msgs_ap, et = consume()
                            oh = pbo.tile([P, P], dt.bfloat16, tag="oh")
                            nc.vector.tensor_tensor(
                                out=oh[:],
                                in0=rsb[:, et: et + 1].to_broadcast([P, P]),
                                in1=iota_sb[:],
                                op=Alu.is_equal,
                            )
                            nc.tensor.matmul(
                                ps[:], lhsT=oh[:], rhs=msgs_ap,
                                start=(j == 0), stop=(j == total - 1),
                            )
                        ysl = y_acc[:, t * P: (t + 1) * P]
                        if s == 0:
                            nc.vector.tensor_copy(ysl, ps[:])
                        else:
                            nc.vector.tensor_tensor(
                                out=ysl, in0=ysl, in1=ps[:], op=Alu.add)
                        if s == S - 1:
                            # finalize: mean + transpose for the GRU phase
                            y_sb = pbo.tile([P, MSG], dt.bfloat16, tag="y")
                            nc.vector.tensor_tensor(
                                out=y_sb[:],
                                in0=inv_sb[:, t: t + 1].to_broadcast([P, MSG]),
                                in1=ysl, op=Alu.mult,
                            )
                            yt_ps = pbp.tile([P, P], dt.bfloat16, tag="ytp")
                            nc.tensor.transpose(yt_ps[:], y_sb[:], ident_bf[:])
                            nc.vector.tensor_copy(
                                yT_own[:, t * P: (t + 1) * P], yt_ps[:])

                # interleave: emit A(s), with B(s-1) following so B(s-1)'s
                # gathers run while A(s) computes on the tensor engine
                emit_A(0)
                for s in range(1, S):
                    emit_A(s)
                    emit_B(s - 1)
                emit_B(S - 1)

            # ---------------- phase C: GRU ----------------
            with (
                tc.tile_pool(name="pc", bufs=2) as pc,
                tc.tile_pool(name="pcs", bufs=3) as pcs,
                tc.tile_pool(name="pcp", bufs=2, space="PSUM") as pcp,
                tc.tile_pool(name="pcp2", bufs=2, space="PSUM") as pcp2,
            ):
                for ch in range(SHARD_PAD // CHUNK):
                    csl = slice(ch * CHUNK, (ch + 1) * CHUNK)
                    z_in = pc.tile([P, CHUNK // P, OUT_F], dt.float32, tag="z_in")
                    nc.sync.dma_start(
                        z_in[:], z_own_d[csl, :].rearrange("(t p) f -> p t f", p=P))
                    zT = pc.tile([P, 2, CHUNK], dt.bfloat16, tag="zT")
                    nc.sync.dma_start(
                        zT[:], zt_own_d.rearrange("(k p) n -> p k n", p=P)[:, :, csl])
                    xT2 = pc.tile([P, 2, CHUNK], dt.bfloat16, tag="xT2")
                    nc.sync.dma_start(
                        xT2[:], xt_own_d.rearrange("(k p) n -> p k n", p=P)[:, :, csl])
                    ho_sb = pc.tile([P, CHUNK // P, OUT_F], dt.float32, tag="ho")
                    for t4 in range(CHUNK // P):
                        tg = ch * (CHUNK // P) + t4
                        tsl = slice(t4 * P, (t4 + 1) * P)
                        ysl = slice(tg * P, (tg + 1) * P)

                        # fused r|u gates: gi+gh accumulated in one 512-wide psum
                        ps_ru = pcp.tile([P, 2 * OUT_F], dt.float32, tag="ps_ru")
                        for kk in range(2):
                            nc.tensor.matmul(
                                ps_ru[:], lhsT=xT2[:, kk, tsl],
                                rhs=wiht_sb[:, kk, 0: 2 * OUT_F],
                                start=(kk == 0), stop=False)
                        nc.tensor.matmul(
                            ps_ru[:], lhsT=yT_own[:, ysl],
                            rhs=wiht_sb[:, 2, 0: 2 * OUT_F],
                            start=False, stop=False)
                        for kk in range(2):
                            nc.tensor.matmul(
                                ps_ru[:], lhsT=zT[:, kk, tsl],
                                rhs=whht_sb[:, kk, 0: 2 * OUT_F],
                                start=False, stop=(kk == 1))

                        # n gate halves: [gi_n | gh_n] in one bank
                        ps_n = pcp2.tile([P, 2 * OUT_F], dt.float32, tag="ps_n")
                        nsl = slice(2 * OUT_F, 3 * OUT_F)
                        for kk in range(2):
                            nc.tensor.matmul(
                                ps_n[:, 0:OUT_F], lhsT=xT2[:, kk, tsl],
                                rhs=wiht_sb[:, kk, nsl],
                                start=(kk == 0), stop=False)
                        nc.tensor.matmul(
                            ps_n[:, 0:OUT_F], lhsT=yT_own[:, ysl],
                            rhs=wiht_sb[:, 2, nsl], start=False, stop=True)
                        for kk in range(2):
                            nc.tensor.matmul(
                                ps_n[:, OUT_F: 2 * OUT_F], lhsT=zT[:, kk, tsl],
                                rhs=whht_sb[:, kk, nsl],
                                start=(kk == 0), stop=(kk == 1))

                        r_sb = pcs.tile([P, OUT_F], dt.float32, tag="r")
                        nc.scalar.activation(r_sb[:], ps_ru[:, 0:OUT_F], Act.Sigmoid)
                        u_sb = pcs.tile([P, OUT_F], dt.float32, tag="u")
                        nc.scalar.activation(u_sb[:], ps_ru[:, OUT_F: 2 * OUT_F],
                                             Act.Sigmoid)
                        t1 = pcs.tile([P, OUT_F], dt.float32, tag="t1")
                        nc.vector.tensor_tensor(
                            out=t1[:], in0=r_sb[:], in1=ps_n[:, OUT_F: 2 * OUT_F],
                            op=Alu.mult)
                        t2 = pcs.tile([P, OUT_F], dt.float32, tag="t2")
                        nc.vector.tensor_tensor(
                            out=t2[:], in0=t1[:], in1=ps_n[:, 0:OUT_F], op=Alu.add)
                        # tanh(v) = 2*sigmoid(2v) - 1
                        s_sb = pcs.tile([P, OUT_F], dt.float32, tag="s")
                        nc.scalar.activation(s_sb[:], t2[:], Act.Sigmoid, scale=2.0)
                        nng = pcs.tile([P, OUT_F], dt.float32, tag="nng")
                        nc.vector.tensor_scalar(
                            nng[:], s_sb[:], 2.0, -1.0, Alu.mult, Alu.add)
                        d_sb = pcs.tile([P, OUT_F], dt.float32, tag="d")
                        nc.vector.tensor_tensor(
                            out=d_sb[:], in0=z_in[:, t4, :], in1=nng[:],
                            op=Alu.subtract)
                        e_sb = pcs.tile([P, OUT_F], dt.float32, tag="e")
                        nc.vector.tensor_tensor(
                            out=e_sb[:], in0=u_sb[:], in1=d_sb[:], op=Alu.mult)
                        nc.vector.tensor_tensor(
                            out=ho_sb[:, t4, :], in0=nng[:], in1=e_sb[:], op=Alu.add)
                    nc.sync.dma_start(
                        hout[csl, :].rearrange("(t p) f -> p t f", p=P), ho_sb[:])

            if debug:
                nc.sync.dma_start(dbg_yt[:, :], yT_own[:])
    return nc


# ---------------------------------------------------------------- entry point

LAST_RESULTS = None  # set when KERNEL_TRACE=1 (used by test.py for timing)


def kernel(**inputs):
    import os

    from concourse.bass_utils import run_bass_kernel_spmd

    in_maps, T = _prep(inputs)
    nc = _build(T, debug=bool(os.environ.get("KERNEL_DEBUG")))
    nc.finalize()
    trace = bool(os.environ.get("KERNEL_TRACE"))
    res = run_bass_kernel_spmd(
        nc, in_maps, core_ids=list(range(N_CORES)), trace=trace
    )
    if trace:
        global LAST_RESULTS
        LAST_RESULTS = res
    out = np.empty((N_REAL, OUT_F), np.float32)
    for k in range(N_CORES):
        out[k * SHARD_REAL: (k + 1) * SHARD_REAL] = res.results[k]["hout"][
            :SHARD_REAL]
    return (out, out)


# revision 11
# speedup vs baseline: 1.2610x; 1.2610x over previous
"""Trainium2 Bass kernel for ContinuousMessagePassing (GNN message passing).

Math (per reference):
    h   = relu(x @ W1.T)                 # [N, 256]
    m   = relu(h @ W2.T)                 # [N, 128]
    y   = segment_mean(m[src], dst, N)   # [N, 128]  (0 for isolated nodes)
    gi  = [x, y] @ W_ih.T ; gh = z @ W_hh.T
    r, u = sigmoid(gi_r + gh_r), sigmoid(gi_u + gh_u)
    n   = tanh(gi_n + r * gh_n)
    out = (1 - u) * n + u * z

Distribution: nodes sharded across 8 cores; the m-table computation is
replicated on every core, split into S=4 source segments so the edge
gather/scatter for segment s overlaps the table compute of segment s+1.
Each core processes the edges whose dst lands in its shard (host buckets,
sorts per (segment, dst-tile, src) and pads to a shared static schedule).

Tensor-engine specifics: operands are pre-transposed on the host (no
on-chip layout transposes); matmuls are issued round-robin across 4
independent PSUM accumulation chains so fill/drain pipelines (a single
accumulation chain serializes the PE array).  Biases are all zeros per
the problem spec and are folded out.
"""

import numpy as np
import ml_dtypes

BF16 = ml_dtypes.bfloat16

# ---------------------------------------------------------------- config

P = 128          # partitions
CHUNK = 512      # nodes per matmul chunk
S = 4            # source segments (phase A/B interleave granularity)
GT = 4           # edge-tiles (of 128 edges) per dma_gather call
NQ = 4           # SWDGE queues
MSG_BUFS = 24    # gather groups in flight

N_REAL = 50000
N_CORES = 8
SHARD_REAL = 6250
SHARD_PAD = 6656          # 52 tiles of 128
NPAD = N_CORES * SHARD_PAD    # 53248
SEG = NPAD // S               # 13312 rows per source segment (int16-safe)
NT = SHARD_PAD // P           # 52 node tiles per shard
IN_F = 256
MSG = 128
HID = 256
OUT_F = 256


class _Cfg:  # kept for test.py compatibility
    n_cores = N_CORES
    n_real = N_REAL
    shard_real = SHARD_REAL
    shard_pad = SHARD_PAD
    npad = NPAD
    nt = NT


CFG8 = _Cfg()


# ---------------------------------------------------------------- host prep

def _wrap_idx16(idx_flat):
    """[n] int array -> [128, n//16] int16 in the dma_gather layout:
    position i lives at [i % 16, i // 16], replicated across the 8 groups
    of 16 partitions (one copy per Q7 core)."""
    n = idx_flat.shape[0]
    a = np.ascontiguousarray(idx_flat.reshape(n // 16, 16).T).astype(np.int16)
    return np.ascontiguousarray(np.tile(a, (8, 1)))


def _prep(inputs):
    x = np.asarray(inputs["x"], np.float32)
    z = np.asarray(inputs["z"], np.float32)
    src = np.asarray(inputs["src"], np.int64)
    dst = np.asarray(inputs["dst"], np.int64)

    # padded node arrays, shard k at rows [k*SHARD_PAD, k*SHARD_PAD+SHARD_REAL)
    xp = np.zeros((NPAD, IN_F), dtype=BF16)
    zp = np.zeros((NPAD, OUT_F), dtype=np.float32)
    for k in range(N_CORES):
        xp[k * SHARD_PAD: k * SHARD_PAD + SHARD_REAL] = x[
            k * SHARD_REAL: (k + 1) * SHARD_REAL]
        zp[k * SHARD_PAD: k * SHARD_PAD + SHARD_REAL] = z[
            k * SHARD_REAL: (k + 1) * SHARD_REAL]
    xt = np.ascontiguousarray(xp.T)                       # [256, NPAD] bf16

    w1t = np.ascontiguousarray(np.asarray(inputs["W1"], np.float32).T).astype(BF16)
    w2t = np.ascontiguousarray(np.asarray(inputs["W2"], np.float32).T).astype(BF16)
    wiht = np.ascontiguousarray(np.asarray(inputs["W_ih"], np.float32).T).astype(BF16)
    whht = np.ascontiguousarray(np.asarray(inputs["W_hh"], np.float32).T).astype(BF16)

    src_pad = (src // SHARD_REAL) * SHARD_PAD + src % SHARD_REAL
    seg = src_pad // SEG
    idx_in_seg = src_pad % SEG
    owner = dst // SHARD_REAL
    dloc = dst - owner * SHARD_REAL
    tile_id = dloc // P
    rel = dloc % P

    # per (core, seg, tile) counts -> shared schedule T[s][t]
    cnt = np.zeros((N_CORES, S, NT), np.int64)
    for k in range(N_CORES):
        sel = owner == k
        np.add.at(cnt[k], (seg[sel], tile_id[sel]), 1)
    T = ((cnt.max(axis=0) + P - 1) // P).astype(np.int64)     # [S, NT]
    # tiles past the real-node region (SHARD_REAL/P) legitimately have no
    # edges; their y is garbage and their outputs are discarded pad rows
    assert (T[:, : SHARD_REAL // P] > 0).all()
    for s in range(S):
        T[s, -1] += (-int(T[s].sum())) % GT
    stot = [int(T[s].sum()) for s in range(S)]

    in_maps = []
    for k in range(N_CORES):
        sel = np.nonzero(owner == k)[0]
        order = np.lexsort((idx_in_seg[sel], tile_id[sel], seg[sel]))
        esel = sel[order]
        sseg = seg[esel]
        stid = tile_id[esel]

        idx16_s, rel_s = [], []
        for s in range(S):
            idx_stream = np.zeros(stot[s] * P, np.int64)
            rel_stream = np.full(stot[s] * P, -1.0, np.float32)
            off = 0
            for t in range(NT):
                e = esel[(sseg == s) & (stid == t)]
                c = e.shape[0]
                idx_stream[off: off + c] = idx_in_seg[e]
                rel_stream[off: off + c] = rel[e]
                off += int(T[s, t]) * P
            blocks = [
                _wrap_idx16(idx_stream[g * GT * P: (g + 1) * GT * P])
                for g in range(stot[s] // GT)
            ]
            idx16_s.append(np.concatenate(blocks, axis=1))
            rel_s.append(np.ascontiguousarray(
                rel_stream.reshape(stot[s], P).T).astype(BF16))

        cnt_nodes = np.bincount(dloc[owner == k], minlength=SHARD_PAD)
        inv = (1.0 / np.maximum(cnt_nodes, 1)).astype(np.float32)
        inv2 = np.ascontiguousarray(inv.reshape(NT, P).T)          # [128, NT]

        r0 = k * SHARD_PAD
        im = {
            "xt": xt,
            "xt_own": np.ascontiguousarray(xt[:, r0: r0 + SHARD_PAD]),
            "zt_own": np.ascontiguousarray(zp[r0: r0 + SHARD_PAD].T).astype(BF16),
            "z_own": zp[r0: r0 + SHARD_PAD],
            "w1t": w1t,
            "w2t": w2t,
            "wiht": wiht,
            "whht": whht,
            "invcnt": inv2,
            "iota_c": np.tile(np.arange(P, dtype=np.float32), (P, 1)).astype(BF16),
            "ident_bf": np.eye(P, dtype=np.float32).astype(BF16),
        }
        for s in range(S):
            im[f"idx_s{s}"] = idx16_s[s]
            im[f"rel_s{s}"] = rel_s[s]
        in_maps.append(im)
    return in_maps, T


# ---------------------------------------------------------------- device program

def _build(T, debug=False):
    import concourse.bass as bass  # noqa: F401
    import concourse.tile as tile
    from concourse import bacc, mybir

    dt = mybir.dt
    Act = mybir.ActivationFunctionType
    Alu = mybir.AluOpType

    stot = [int(T[s].sum()) for s in range(S)]
    CPS = (NPAD // CHUNK) // S          # phase-A chunks per segment (26)

    nc = bacc.Bacc(None, num_swdge_queues=NQ)

    xt_d = nc.dram_tensor("xt", [IN_F, NPAD], dt.bfloat16, kind="ExternalInput")
    xt_own_d = nc.dram_tensor("xt_own", [IN_F, SHARD_PAD], dt.bfloat16, kind="ExternalInput")
    zt_own_d = nc.dram_tensor("zt_own", [OUT_F, SHARD_PAD], dt.bfloat16, kind="ExternalInput")
    z_own_d = nc.dram_tensor("z_own", [SHARD_PAD, OUT_F], dt.float32, kind="ExternalInput")
    w1t_d = nc.dram_tensor("w1t", [IN_F, HID], dt.bfloat16, kind="ExternalInput")
    w2t_d = nc.dram_tensor("w2t", [HID, MSG], dt.bfloat16, kind="ExternalInput")
    wiht_d = nc.dram_tensor("wiht", [IN_F + MSG, 3 * OUT_F], dt.bfloat16, kind="ExternalInput")
    whht_d = nc.dram_tensor("whht", [OUT_F, 3 * OUT_F], dt.bfloat16, kind="ExternalInput")
    idx_d = [nc.dram_tensor(f"idx_s{s}", [P, stot[s] * 8], dt.int16, kind="ExternalInput")
             for s in range(S)]
    rel_d = [nc.dram_tensor(f"rel_s{s}", [P, stot[s]], dt.bfloat16, kind="ExternalInput")
             for s in range(S)]
    inv_d = nc.dram_tensor("invcnt", [P, NT], dt.float32, kind="ExternalInput")
    iota_d = nc.dram_tensor("iota_c", [P, P], dt.bfloat16, kind="ExternalInput")
    identb_d = nc.dram_tensor("ident_bf", [P, P], dt.bfloat16, kind="ExternalInput")
    hout = nc.dram_tensor("hout", [SHARD_PAD, OUT_F], dt.float32, kind="ExternalOutput")
    if debug:
        dbg_yt = nc.dram_tensor("dbg_yt", [P, SHARD_PAD], dt.bfloat16, kind="ExternalOutput")
    # m table, 128-elem (256B) bf16 rows, split in S segment tensors so the
    # segment-s gathers only depend on segment-s writes
    m_seg = [nc.dram_tensor(f"m_seg{s}", [SEG, MSG], dt.bfloat16) for s in range(S)]

    with tile.TileContext(nc) as tc:
        with tc.tile_pool(name="persist", bufs=1) as pers:
            w1t_sb = pers.tile([P, 2, HID], dt.bfloat16)
            nc.sync.dma_start(w1t_sb[:], w1t_d[:, :].rearrange("(k p) n -> p k n", p=P))
            w2t_sb = pers.tile([P, 2, MSG], dt.bfloat16)
            nc.sync.dma_start(w2t_sb[:], w2t_d[:, :].rearrange("(k p) n -> p k n", p=P))
            wiht_sb = pers.tile([P, 3, 3 * OUT_F], dt.bfloat16)
            nc.sync.dma_start(wiht_sb[:], wiht_d[:, :].rearrange("(k p) n -> p k n", p=P))
            whht_sb = pers.tile([P, 2, 3 * OUT_F], dt.bfloat16)
            nc.sync.dma_start(whht_sb[:], whht_d[:, :].rearrange("(k p) n -> p k n", p=P))
            idx_sb = []
            rel_sb = []
            for s in range(S):
                isb = pers.tile([P, stot[s] * 8], dt.int16)
                nc.sync.dma_start(isb[:], idx_d[s][:, :])
                idx_sb.append(isb)
                rsb = pers.tile([P, stot[s]], dt.bfloat16)
                nc.sync.dma_start(rsb[:], rel_d[s][:, :])
                rel_sb.append(rsb)
            inv_sb = pers.tile([P, NT], dt.float32)
            nc.sync.dma_start(inv_sb[:], inv_d[:, :])
            iota_sb = pers.tile([P, P], dt.bfloat16)
            nc.sync.dma_start(iota_sb[:], iota_d[:, :])
            ident_bf = pers.tile([P, P], dt.bfloat16)
            nc.sync.dma_start(ident_bf[:], identb_d[:, :])

            y_acc = pers.tile([P, SHARD_PAD], dt.float32)    # per-dst partial sums
            yT_own = pers.tile([P, SHARD_PAD], dt.bfloat16)  # y transposed for GRU

            with (
                tc.tile_pool(name="pa", bufs=2) as pa,
                tc.tile_pool(name="pap", bufs=2, space="PSUM") as pap,
                tc.tile_pool(name="pb", bufs=MSG_BUFS) as pb,
                tc.tile_pool(name="pbo", bufs=8) as pbo,
                tc.tile_pool(name="pbp", bufs=2, space="PSUM") as pbp,
            ):
                # ---------------- phase A: m-table for one source segment ----
                def emit_A(s):
                    for cc in range(CPS):
                        c = s * CPS + cc
                        xT = pa.tile([P, 2, CHUNK], dt.bfloat16, tag="xT")
                        nc.sync.dma_start(
                            xT[:],
                            xt_d.rearrange("(k p) n -> p k n", p=P)[
                                :, :, c * CHUNK: (c + 1) * CHUNK],
                        )
                        # mm1: 4 independent 256-col accumulation chains
                        # (mh = output hid half, hh = node half) issued
                        # round-robin so the PE array pipelines
                        hp_all = pap.tile([P, 4, CHUNK // 2], dt.float32, tag="hp")
                        hp = [hp_all[:, i, :] for i in range(4)]
                        for kk in range(2):
                            for mh in range(2):
                                for hh in range(2):
                                    nc.tensor.matmul(
                                        hp[mh * 2 + hh],
                                        lhsT=w1t_sb[:, kk, mh * P: (mh + 1) * P],
                                        rhs=xT[:, kk, hh * 256: (hh + 1) * 256],
                                        start=(kk == 0), stop=(kk == 1),
                                    )
                        hT = pa.tile([P, 2, CHUNK], dt.bfloat16, tag="hT")
                        for mh in range(2):
                            for hh in range(2):
                                dst = hT[:, mh, hh * 256: (hh + 1) * 256]
                                src = hp[mh * 2 + hh]
                                if hh == 0:
                                    nc.scalar.activation(dst, src, Act.Relu)
                                else:
                                    nc.vector.tensor_scalar(
                                        dst, src, 0.0, 0.0, Alu.max, Alu.add)
                        # mm2: 4 independent chains (one per node subtile)
                        mp_all = pap.tile([P, 4, MSG], dt.float32, tag="mp")
                        mp = [mp_all[:, i, :] for i in range(4)]
                        for kk in range(2):
                            for t4 in range(4):
                                nc.tensor.matmul(
                                    mp[t4],
                                    lhsT=hT[:, kk, t4 * P: (t4 + 1) * P],
                                    rhs=w2t_sb[:, kk, :],
                                    start=(kk == 0), stop=(kk == 1),
                                )
                        m_sb = pa.tile([P, CHUNK // P, MSG], dt.bfloat16, tag="m_sb")
                        for t4 in range(4):
                            if t4 % 2 == 0:
                                nc.scalar.activation(
                                    m_sb[:, t4, :], mp[t4], Act.Relu)
                            else:
                                nc.vector.tensor_scalar(
                                    m_sb[:, t4, :], mp[t4], 0.0, 0.0,
                                    Alu.max, Alu.add)
                        nc.sync.dma_start(
                            m_seg[s][cc * CHUNK: (cc + 1) * CHUNK, :].rearrange(
                                "(t p) f -> p t f", p=P),
                            m_sb[:],
                        )

                # ---------------- phase B: gather + segment reduce, one pass --
                gq = [0]

                def emit_B(s):
                    m_ap = m_seg[s][:, :]
                    isb = idx_sb[s]
                    rsb = rel_sb[s]
                    state = {"et": 0, "msgs": None}

                    def consume():
                        et = state["et"]
                        g, slot = divmod(et, GT)
                        if slot == 0:
                            msgs = pb.tile([P, GT, MSG], dt.bfloat16, tag="msgs")
                            nc.gpsimd.dma_gather(
                                msgs[:], m_ap,
                                isb[:, g * GT * 8: (g + 1) * GT * 8],
                                GT * P, GT * P, MSG,
                                queue_num=gq[0] % NQ,
                            )
                            gq[0] += 1
                            state["msgs"] = msgs
                        state["et"] = et + 1
                        return state["msgs"][:, slot, :], et

                    for t in range(NT):
                        total = int(T[s, t])
                        if total == 0:
                            continue
                        ps = pbp.tile([P, MSG], dt.float32, tag="ps")
                        for j in range(total):
                            msgs_ap, et = consume()
                            oh = pbo.tile([P, P], dt.bfloat16, tag="oh")
                            nc.vector.tensor_tensor(
                                out=oh[:],
                                in0=rsb[:, et: et + 1].to_broadcast([P, P]),
                                in1=iota_sb[:],
                                op=Alu.is_equal,
                            )
                            nc.tensor.matmul(
                                ps[:], lhsT=oh[:], rhs=msgs_ap,
                                start=(j == 0), stop=(j == total - 1),
                            )
                        ysl = y_acc[:, t * P: (t + 1) * P]
                        if s == 0:
                            nc.vector.tensor_copy(ysl, ps[:])
                        else:
                            nc.vector.tensor_tensor(
                                out=ysl, in0=ysl, in1=ps[:], op=Alu.add)
                        if s == S - 1:
                            # finalize: mean + transpose for the GRU phase
                            y_sb = pbo.tile([P, MSG], dt.bfloat16, tag="y")
                            nc.vector.tensor_tensor(
                                out=y_sb[:],
                                in0=inv_sb[:, t: t + 1].to_broadcast([P, MSG]),
                                in1=ysl, op=Alu.mult,
                            )
                            yt_ps = pbp.tile([P, 2, P], dt.bfloat16, tag="ps")
                            nc.tensor.transpose(yt_ps[:, 0, :], y_sb[:], ident_bf[:])
                            nc.vector.tensor_copy(
                                yT_own[:, t * P: (t + 1) * P], yt_ps[:, 0, :])

                # interleave: emit A(s), with B(s-1) following so B(s-1)'s
                # gathers run while A(s) computes on the tensor engine
                emit_A(0)
                for s in range(1, S):
                    emit_A(s)
                    emit_B(s - 1)
                emit_B(S - 1)

            # ---------------- phase C: GRU ----------------
            with (
                tc.tile_pool(name="pc", bufs=2) as pc,
                tc.tile_pool(name="pcs", bufs=3) as pcs,
                tc.tile_pool(name="pcp", bufs=2, space="PSUM") as pcp,
            ):
                rsl = slice(0, OUT_F)
                usl = slice(OUT_F, 2 * OUT_F)
                nsl = slice(2 * OUT_F, 3 * OUT_F)
                for ch in range(SHARD_PAD // CHUNK):
                    csl = slice(ch * CHUNK, (ch + 1) * CHUNK)
                    z_in = pc.tile([P, CHUNK // P, OUT_F], dt.float32, tag="z_in")
                    nc.sync.dma_start(
                        z_in[:], z_own_d[csl, :].rearrange("(t p) f -> p t f", p=P))
                    zT = pc.tile([P, 2, CHUNK], dt.bfloat16, tag="zT")
                    nc.sync.dma_start(
                        zT[:], zt_own_d.rearrange("(k p) n -> p k n", p=P)[:, :, csl])
                    xT2 = pc.tile([P, 2, CHUNK], dt.bfloat16, tag="xT2")
                    nc.sync.dma_start(
                        xT2[:], xt_own_d.rearrange("(k p) n -> p k n", p=P)[:, :, csl])
                    ho_sb = pc.tile([P, CHUNK // P, OUT_F], dt.float32, tag="ho")
                    for t4 in range(CHUNK // P):
                        tg = ch * (CHUNK // P) + t4
                        tsl = slice(t4 * P, (t4 + 1) * P)
                        ysl = slice(tg * P, (tg + 1) * P)

                        # 4 independent 256-col chains: r, u, gi_n, gh_n
                        pr = pcp.tile([P, OUT_F], dt.float32, tag="pr")
                        pu = pcp.tile([P, OUT_F], dt.float32, tag="pu")
                        pni = pcp.tile([P, OUT_F], dt.float32, tag="pni")
                        phn = pcp.tile([P, OUT_F], dt.float32, tag="phn")
                        for kk in range(2):
                            nc.tensor.matmul(
                                pr[:], lhsT=xT2[:, kk, tsl],
                                rhs=wiht_sb[:, kk, rsl],
                                start=(kk == 0), stop=False)
                            nc.tensor.matmul(
                                pu[:], lhsT=xT2[:, kk, tsl],
                                rhs=wiht_sb[:, kk, usl],
                                start=(kk == 0), stop=False)
                            nc.tensor.matmul(
                                pni[:], lhsT=xT2[:, kk, tsl],
                                rhs=wiht_sb[:, kk, nsl],
                                start=(kk == 0), stop=False)
                            nc.tensor.matmul(
                                phn[:], lhsT=zT[:, kk, tsl],
                                rhs=whht_sb[:, kk, nsl],
                                start=(kk == 0), stop=(kk == 1))
                        nc.tensor.matmul(
                            pr[:], lhsT=yT_own[:, ysl],
                            rhs=wiht_sb[:, 2, rsl], start=False, stop=False)
                        nc.tensor.matmul(
                            pu[:], lhsT=yT_own[:, ysl],
                            rhs=wiht_sb[:, 2, usl], start=False, stop=False)
                        nc.tensor.matmul(
                            pni[:], lhsT=yT_own[:, ysl],
                            rhs=wiht_sb[:, 2, nsl], start=False, stop=True)
                        for kk in range(2):
                            nc.tensor.matmul(
                                pr[:], lhsT=zT[:, kk, tsl],
                                rhs=whht_sb[:, kk, rsl],
                                start=False, stop=(kk == 1))
                            nc.tensor.matmul(
                                pu[:], lhsT=zT[:, kk, tsl],
                                rhs=whht_sb[:, kk, usl],
                                start=False, stop=(kk == 1))

                        r_sb = pcs.tile([P, OUT_F], dt.float32, tag="r")
                        nc.scalar.activation(r_sb[:], pr[:], Act.Sigmoid)
                        u_sb = pcs.tile([P, OUT_F], dt.float32, tag="u")
                        nc.scalar.activation(u_sb[:], pu[:], Act.Sigmoid)
                        t1 = pcs.tile([P, OUT_F], dt.float32, tag="t1")
                        nc.vector.tensor_tensor(
                            out=t1[:], in0=r_sb[:], in1=phn[:], op=Alu.mult)
                        t2 = pcs.tile([P, OUT_F], dt.float32, tag="t2")
                        nc.vector.tensor_tensor(
                            out=t2[:], in0=t1[:], in1=pni[:], op=Alu.add)
                        # tanh(v) = 2*sigmoid(2v) - 1
                        s_sb = pcs.tile([P, OUT_F], dt.float32, tag="s")
                        nc.scalar.activation(s_sb[:], t2[:], Act.Sigmoid, scale=2.0)
                        nng = pcs.tile([P, OUT_F], dt.float32, tag="nng")
                        nc.vector.tensor_scalar(
                            nng[:], s_sb[:], 2.0, -1.0, Alu.mult, Alu.add)
                        d_sb = pcs.tile([P, OUT_F], dt.float32, tag="d")
                        nc.vector.tensor_tensor(
                            out=d_sb[:], in0=z_in[:, t4, :], in1=nng[:],
                            op=Alu.subtract)
                        e_sb = pcs.tile([P, OUT_F], dt.float32, tag="e")
                        nc.vector.tensor_tensor(
                            out=e_sb[:], in0=u_sb[:], in1=d_sb[:], op=Alu.mult)
                        nc.vector.tensor_tensor(
                            out=ho_sb[:, t4, :], in0=nng[:], in1=e_sb[:], op=Alu.add)
                    nc.sync.dma_start(
                        hout[csl, :].rearrange("(t p) f -> p t f", p=P), ho_sb[:])

            if debug:
                nc.sync.dma_start(dbg_yt[:, :], yT_own[:])
    return nc


# ---------------------------------------------------------------- entry point

LAST_RESULTS = None  # set when KERNEL_TRACE=1 (used by test.py for timing)


def kernel(**inputs):
    import os

    from concourse.bass_utils import run_bass_kernel_spmd

    in_maps, T = _prep(inputs)
    nc = _build(T, debug=bool(os.environ.get("KERNEL_DEBUG")))
    nc.finalize()
    trace = bool(os.environ.get("KERNEL_TRACE"))
    res = run_bass_kernel_spmd(
        nc, in_maps, core_ids=list(range(N_CORES)), trace=trace
    )
    if trace:
        global LAST_RESULTS
        LAST_RESULTS = res
    out = np.empty((N_REAL, OUT_F), np.float32)
    for k in range(N_CORES):
        out[k * SHARD_REAL: (k + 1) * SHARD_REAL] = res.results[k]["hout"][
            :SHARD_REAL]
    return (out, out)
